# revision 1
# baseline (speedup 1.0000x reference)
"""CrossAndCompress Trainium2 kernel.

Reference computation (per row r of the batch):
    a_r = enc_item[r] . theta_vv        b_r = enc_user[r] . theta_ev
    c_r = enc_item[r] . theta_ve        d_r = enc_user[r] . theta_ee
    v_out[r] = enc_user[r] * a_r + enc_item[r] * b_r + beta_v
    e_out[r] = enc_user[r] * c_r + enc_item[r] * d_r + beta_e

Sharding: pure data parallel — batch dim (16384) split across 8 NeuronCores
(2048 rows each); the tiny theta/beta vectors are replicated (host-side
pre-broadcast to 128 partitions so DVE can consume them directly).

Per-core pipeline over 16 tiles of [128 rows x 1024]:
  - DMA in user/item tiles (natural layout: rows on partitions, contiguous HBM)
  - 4 fused multiply+reduce dots on VectorE (affine_mul_reduce custom DVE op:
    one pass each computes the product AND its free-axis sum)
  - item-scaled products p2 = it*b, p4 = it*d on ScalarE (activation with
    per-partition scale AP)
  - fused scale+add on VectorE (scalar_tensor_tensor):
    v = u*a + p2, e = u*c + p4
  - DMA out (no PSUM, TensorE/GpSimd unused: fp32 PE matmul and GpSimd
    elementwise both measured slower than the DVE passes they would replace)
"""

import numpy as np

B, D = 16384, 1024
N_CORES = 8
ROWS_PER_CORE = B // N_CORES  # 2048
TILE_P = 128
N_TILES = ROWS_PER_CORE // TILE_P  # 16

_PROGRAM_CACHE: dict = {}


def _build_program(with_beta: bool):
    import concourse.bass as bass  # noqa: F401
    import concourse.mybir as mybir
    import concourse.tile as tile
    from concourse import bacc

    f32 = mybir.dt.float32
    AF = mybir.ActivationFunctionType
    OP = mybir.AluOpType

    nc = bacc.Bacc(
        "TRN2",
        target_bir_lowering=False,
        debug=False,
        enable_asserts=False,
        num_devices=N_CORES,
    )

    u_h = nc.dram_tensor(
        "enc_user", [ROWS_PER_CORE, D], f32, kind="ExternalInput"
    ).ap()
    i_h = nc.dram_tensor(
        "enc_item", [ROWS_PER_CORE, D], f32, kind="ExternalInput"
    ).ap()
    th_h = nc.dram_tensor("thetas", [TILE_P, 4 * D], f32, kind="ExternalInput").ap()
    if with_beta:
        be_h = nc.dram_tensor("betas", [TILE_P, 2 * D], f32, kind="ExternalInput").ap()
    v_h = nc.dram_tensor("v_out", [ROWS_PER_CORE, D], f32, kind="ExternalOutput").ap()
    e_h = nc.dram_tensor("e_out", [ROWS_PER_CORE, D], f32, kind="ExternalOutput").ap()

    with tile.TileContext(nc) as tc:
        with (
            tc.tile_pool(name="const", bufs=1) as cpool,
            tc.tile_pool(name="io", bufs=4) as io,
            tc.tile_pool(name="work", bufs=4) as work,
        ):
            # DMA packets stripe across all 16 HW queues automatically, so
            # fewest-triggers wins. Theta loads as one DMA per slice, in
            # first-use order, so the first dot waits ~0.5MB, not 2MB.
            th = cpool.tile([TILE_P, 4 * D], f32)
            for k in (0, 1, 2, 3):
                nc.sync.dma_start(
                    th[:, k * D : (k + 1) * D], th_h[:, k * D : (k + 1) * D]
                )
            if with_beta:
                betas = cpool.tile([TILE_P, 2 * D], f32)
                nc.sync.dma_start(betas[:], be_h[:, :])

            # theta layout along free dim: [t_vv | t_ev | t_ve | t_ee]
            t_sl = [th[:, k * D : (k + 1) * D] for k in range(4)]

            for i in range(N_TILES):
                rows = slice(i * TILE_P, (i + 1) * TILE_P)
                u = io.tile([TILE_P, D], f32, tag="u")
                it = io.tile([TILE_P, D], f32, tag="it")
                nc.sync.dma_start(u[:], u_h[rows, :])
                nc.sync.dma_start(it[:], i_h[rows, :])

                # dots[:, 0..3] = a, b, c, d
                dots = work.tile([TILE_P, 4], f32, tag="dots")
                for k, src in ((0, it), (1, u), (2, it), (3, u)):
                    scr = work.tile([TILE_P, D], f32, tag="scr")
                    nc.vector.affine_mul_reduce(
                        out=scr[:],
                        accum_out=dots[:, k : k + 1],
                        in0=src[:],
                        in1=t_sl[k],
                        scale=1.0,
                        bias=0.0,
                    )
                d_a, d_b, d_c, d_d = (dots[:, k : k + 1] for k in range(4))

                # item-scaled products on ScalarE: p2 = it*b, p4 = it*d
                p2 = work.tile([TILE_P, D], f32, tag="p2")
                nc.scalar.activation(p2[:], it[:], AF.Copy, bias=0.0, scale=d_b)
                p4 = work.tile([TILE_P, D], f32, tag="p4")
                nc.scalar.activation(p4[:], it[:], AF.Copy, bias=0.0, scale=d_d)

                # fused scale+add on VectorE: v = u*a + p2, e = u*c + p4
                v_sb = io.tile([TILE_P, D], f32, tag="v_sb")
                e_sb = io.tile([TILE_P, D], f32, tag="e_sb")
                nc.vector.scalar_tensor_tensor(
                    out=v_sb[:], in0=u[:], scalar=d_a, in1=p2[:],
                    op0=OP.mult, op1=OP.add)
                nc.vector.scalar_tensor_tensor(
                    out=e_sb[:], in0=u[:], scalar=d_c, in1=p4[:],
                    op0=OP.mult, op1=OP.add)
                if with_beta:
                    v_sb2 = io.tile([TILE_P, D], f32, tag="v_sb2")
                    e_sb2 = io.tile([TILE_P, D], f32, tag="e_sb2")
                    nc.vector.tensor_add(v_sb2[:], v_sb[:], betas[:, 0:D])
                    nc.vector.tensor_add(e_sb2[:], e_sb[:], betas[:, D : 2 * D])
                    v_sb, e_sb = v_sb2, e_sb2
                nc.sync.dma_start(v_h[rows, :], v_sb[:])
                nc.sync.dma_start(e_h[rows, :], e_sb[:])

    nc.compile()
    return nc


def _get_program(with_beta: bool):
    if with_beta not in _PROGRAM_CACHE:
        _PROGRAM_CACHE[with_beta] = _build_program(with_beta)
    return _PROGRAM_CACHE[with_beta]


def _prep_host_inputs(inputs):
    enc_user = np.ascontiguousarray(np.asarray(inputs["enc_user"], dtype=np.float32))
    enc_item = np.ascontiguousarray(np.asarray(inputs["enc_item"], dtype=np.float32))
    assert enc_user.shape == (B, D) and enc_item.shape == (B, D)

    def vec(name):
        return np.asarray(inputs[name], dtype=np.float32).reshape(D)

    thetas = np.concatenate(
        [vec("theta_vv"), vec("theta_ev"), vec("theta_ve"), vec("theta_ee")]
    )
    thetas_b = np.ascontiguousarray(
        np.broadcast_to(thetas[None, :], (TILE_P, 4 * D))
    )
    beta_v, beta_e = vec("beta_v"), vec("beta_e")
    with_beta = bool(np.any(beta_v) or np.any(beta_e))
    betas_b = None
    if with_beta:
        betas_b = np.ascontiguousarray(
            np.broadcast_to(
                np.concatenate([beta_v, beta_e])[None, :], (TILE_P, 2 * D)
            )
        )
    return enc_user, enc_item, thetas_b, betas_b, with_beta


def _make_in_maps(enc_user, enc_item, thetas_b, betas_b, with_beta):
    in_maps = []
    for c in range(N_CORES):
        rows = slice(c * ROWS_PER_CORE, (c + 1) * ROWS_PER_CORE)
        m = {
            "enc_user": np.ascontiguousarray(enc_user[rows]),
            "enc_item": np.ascontiguousarray(enc_item[rows]),
            "thetas": thetas_b,
        }
        if with_beta:
            m["betas"] = betas_b
        in_maps.append(m)
    return in_maps


def run_on_hw(inputs, trace=False):
    """Build/fetch the program, run it SPMD on 8 cores, gather outputs.

    Returns ((v_out, e_out), BassKernelResults).
    """
    import time

    from concourse.bass_utils import run_bass_kernel_spmd

    host = _prep_host_inputs(inputs)
    with_beta = host[-1]
    nc = _get_program(with_beta)
    in_maps = _make_in_maps(*host)
    for attempt in range(3):
        try:
            res = run_bass_kernel_spmd(nc, in_maps, list(range(N_CORES)), trace=trace)
            break
        except Exception:
            if attempt == 2:
                raise
            time.sleep(2.0)
    v = np.concatenate([np.asarray(res.results[c]["v_out"]) for c in range(N_CORES)], axis=0)
    e = np.concatenate([np.asarray(res.results[c]["e_out"]) for c in range(N_CORES)], axis=0)
    return (v, e), res


def kernel(**inputs):
    (v, e), _ = run_on_hw(inputs, trace=False)
    return v, e

